# revision 2
# baseline (speedup 1.0000x reference)
"""ConcatenatedLoRALinearSidecarLayer kernel for 8x TRN2 NeuronCores.

Reference computation (per LoRA branch n, then concat over n on the last dim):
    h_n = x @ down_n.T                      # [M, R]
    y_n = (h_n @ up_n.T + bias_n) * (WEIGHT * scales_n)
    out = concat_n(y_n)                     # [M, N*O]

Strategy (v2 -- DMA-roofline oriented):
  - Data-parallel over tokens M = B*S = 16384 -> 2048 tokens per core.
  - ALL device IO in bf16: x in, down/up weights in, y out. The fp32 result
    is reconstructed on the host (bf16 -> fp32 upcast + bias add). This
    roughly halves HBM traffic vs fp32 (the old roofline): per core
    ~17MB x-in + ~50MB y-out + ~6MB weights ~= 73MB @ ~358 GB/s ~= 205us.
  - bias*weight*scale is folded on the host and added on the HOST, not the
    device: a PSUM-sourced tensor_tensor add runs at 1x DVE mode and was
    costing ~250us/core in the old kernel. Device only ever does
    tensor_copy / activation-Copy out of PSUM.
  - Host-side prep lays x out so every x tile DMA is fully contiguous
    per partition (16KB lines, 4MB transfers); y writes are full token
    rows (24KB lines, 3MB transfers).
  - Per core, per 512-token block:
      phase 1:  hT_n[r, t] += dT_n[d, r].T @ xT[d, t] over 32 d-chunks
                (moving free dim 512, fp32 PSUM accumulate)
      hT copy:  DVE PSUM->SBUF, cast to bf16
      phase 2:  per 128-token sub-block, per branch, per 512-wide o chunk:
                y[t, o] = hT_n[r, t].T @ uT_n[r, o]  (single MM, fp32 PSUM)
      copyback: PSUM->SBUF bf16 casts alternate between DVE tensor_copy
                and ScalarE activation(Copy) so neither engine bottlenecks.
  - All weights (dT, uT) stay resident in SBUF.

Wait-slot legalization: this container's walrus accepts at most 1 sync-wait
per instruction; a JSON post-pass splits excess waits onto same-engine NoOps
with identical blocking semantics.
"""

from contextlib import ExitStack

import numpy as np

import concourse.bass as bass
import concourse.mybir as mybir
import concourse.tile as tile

WEIGHT = 0.8
N_CORES = 8
B, S, D = 4, 4096, 4096
NL, R, O = 3, 128, 4096
M = B * S                    # 16384 tokens total
T = M // N_CORES             # 2048 tokens per core
NR = NL * R                  # 384
NO = NL * O                  # 12288

P = 128                      # SBUF partitions
TB = 512                     # token block (phase-1 moving free dim)
DO = D // P                  # 32 contraction chunks
DH = DO // 2                 # d-chunks per x half-load
OC = 512                     # phase-2 moving free dim / PSUM tile (fp32 bank)

F32 = mybir.dt.float32
BF16 = mybir.dt.bfloat16

MAX_WAITS = 1


def build_nc(t_core: int = T) -> bass.Bass:
    assert t_core % TB == 0
    n_tb = t_core // TB

    nc = bass.Bass("TRN2", target_bir_lowering=False, debug=False)

    xT = nc.dram_tensor("xT", [P, n_tb, 2, DH, TB], BF16, kind="ExternalInput")
    dT = nc.dram_tensor("dT", [P, DO, NR], BF16, kind="ExternalInput")
    uT = nc.dram_tensor("uT", [R, NO], BF16, kind="ExternalInput")
    y = nc.dram_tensor("y", [t_core, NO], BF16, kind="ExternalOutput")

    copy_fn = mybir.ActivationFunctionType.Copy

    with tile.TileContext(nc) as tc, ExitStack() as ctx:
        const = ctx.enter_context(tc.tile_pool(name="const", bufs=1))
        xpool = ctx.enter_context(tc.tile_pool(name="xpool", bufs=4))
        hpool = ctx.enter_context(tc.tile_pool(name="hpool", bufs=2))
        ypool = ctx.enter_context(tc.tile_pool(name="ypool", bufs=3))
        ps_h = ctx.enter_context(tc.tile_pool(name="ps_h", bufs=3, space="PSUM"))
        ps_y = ctx.enter_context(tc.tile_pool(name="ps_y", bufs=4, space="PSUM"))

        # Resident weights
        dT_sb = const.tile([P, DO, NR], BF16, name="dT_sb")
        nc.sync.dma_start(dT_sb[:], dT[:, :, :])
        uT_sb = const.tile([P, NO], BF16, name="uT_sb")
        nc.sync.dma_start(uT_sb[:], uT[:, :])

        for tb in range(n_tb):
            # Load this block's x slice in two halves so MMs start early.
            xts = []
            for h in range(2):
                xt = xpool.tile([P, DH, TB], BF16, tag="xt", name=f"xt{tb}_{h}")
                nc.sync.dma_start(xt[:], xT[:, tb, h, :, :])
                xts.append(xt)

            # Phase 1: hT_n[r, 0:TB] accumulated over all d chunks.
            hps = [
                ps_h.tile([P, TB], F32, tag="hps", name=f"hps{tb}_{n}")
                for n in range(NL)
            ]
            for dc in range(DO):
                xs = xts[dc // DH][:, dc % DH, :]
                for n in range(NL):
                    nc.tensor.matmul(
                        hps[n][:],
                        dT_sb[:, dc, n * R:(n + 1) * R],
                        xs,
                        start=(dc == 0),
                        stop=(dc == DO - 1),
                    )

            hT = hpool.tile([P, NL, TB], BF16, tag="hT", name=f"hT{tb}")
            for n in range(NL):
                nc.vector.tensor_copy(hT[:, n, :], hps[n][:])

            # Phase 2: y[t, o] per 128-token sub-block, per branch, per chunk.
            for th in range(TB // P):
                t0 = tb * TB + th * P
                ysb = ypool.tile([P, NO], BF16, tag="ysb", name=f"ysb{tb}_{th}")
                k = 0
                for n in range(NL):
                    lhs = hT[:, n, th * P:(th + 1) * P]
                    o0 = n * O
                    for oc in range(O // OC):
                        yps = ps_y.tile([P, OC], F32, tag="yps",
                                        name=f"yps{tb}_{th}_{n}_{oc}")
                        nc.tensor.matmul(
                            yps[:],
                            lhs,
                            uT_sb[:, o0 + oc * OC: o0 + (oc + 1) * OC],
                            start=True,
                            stop=True,
                        )
                        dst = ysb[:, o0 + oc * OC: o0 + (oc + 1) * OC]
                        if k % 2 == 0:
                            nc.vector.tensor_copy(dst, yps[:])
                        else:
                            nc.scalar.activation(dst, yps[:], copy_fn)
                        k += 1
                nc.sync.dma_start(y[t0:t0 + P, :], ysb[:])

    _wrap_to_json_with_wait_split(nc)
    return nc


def _legalize_wait_counts(bir: dict) -> None:
    """Split multi-wait instructions: this walrus accepts only ONE sync-wait
    per instruction. Excess waits move onto NoOps inserted just before the
    instruction on the same engine -- identical blocking semantics."""
    n_new = 0
    for fn in bir.get("functions", []):
        for blk in fn.get("blocks", []):
            insts = blk.get("instructions", [])
            out = []
            for inst in insts:
                si = inst.get("sync_info")
                waits = (si or {}).get("on_wait") or []
                if len(waits) > MAX_WAITS:
                    for w in waits[:-1]:
                        nonlocal_name = f"I-waitsplit-{id(inst)}-{n_new}"
                        n_new += 1
                        out.append({
                            "debug": inst.get("debug", 0),
                            "engine": inst["engine"],
                            "ins": [],
                            "name": nonlocal_name,
                            "opcode": "NoOp",
                            "outs": [],
                            "sync_info": {"on_update": [], "on_wait": [w]},
                        })
                    si["on_wait"] = [waits[-1]]
                out.append(inst)
            blk["instructions"] = out


def _wrap_to_json_with_wait_split(nc) -> None:
    import json as _json

    orig = nc.to_json_bytes

    def patched():
        d = _json.loads(orig())
        _legalize_wait_counts(d)
        return _json.dumps(d).encode()

    nc.to_json_bytes = patched


def prep_inputs(x, down, up, bias, scales):
    """Host-side marshalling: transpose + fold scales + bf16 casts.

    Returns (per-core in_maps, bias_w) where bias_w is the host-side
    fp32 bias (already scaled) to add after the device run.
    """
    import ml_dtypes

    x = np.asarray(x, dtype=np.float32)
    down = np.asarray(down, dtype=np.float32)
    up = np.asarray(up, dtype=np.float32)
    bias = np.asarray(bias, dtype=np.float32)
    scales = np.asarray(scales, dtype=np.float32)

    ws = (WEIGHT * scales).astype(np.float32)                       # [NL]
    bias_w = (bias * ws[:, None]).reshape(NO).astype(np.float32)    # [NO]

    # down [NL,R,D] -> [D, NR] -> [do, di, NR] -> [di=128, do, NR]
    dTf = np.ascontiguousarray(
        np.transpose(down, (2, 0, 1)).reshape(DO, P, NR).transpose(1, 0, 2)
    ).astype(ml_dtypes.bfloat16)
    # up [NL,O,R] * ws -> [R, NO]
    uTf = np.ascontiguousarray(
        np.transpose(up * ws[:, None, None], (2, 0, 1)).reshape(R, NO)
    ).astype(ml_dtypes.bfloat16)

    # x -> [D, M] -> per-core [di, tb, half, dh, t] fully-contiguous tiles
    xTf = np.ascontiguousarray(x.reshape(M, D).T).astype(ml_dtypes.bfloat16)

    n_tb = T // TB
    in_maps = []
    for c in range(N_CORES):
        xc = xTf[:, c * T:(c + 1) * T]                    # [D, T]
        xc = xc.reshape(2, DH, P, n_tb, TB)               # [half, dh, di, tb, t]
        xc = np.ascontiguousarray(xc.transpose(2, 3, 0, 1, 4))  # [di,tb,h,dh,t]
        in_maps.append({
            "xT": xc,
            "dT": dTf,
            "uT": uTf,
        })
    return in_maps, bias_w


_CACHED_NC = None


def kernel(x, down, up, bias, scales):
    global _CACHED_NC
    from concourse.bass_utils import run_bass_kernel_spmd

    in_maps, bias_w = prep_inputs(x, down, up, bias, scales)
    if _CACHED_NC is None:
        _CACHED_NC = build_nc(T)
    res = run_bass_kernel_spmd(_CACHED_NC, in_maps, core_ids=list(range(N_CORES)))
    out = np.concatenate(
        [np.asarray(r["y"]).astype(np.float32) for r in res.results], axis=0
    )
    out += bias_w[None, :]
    return out.reshape(B, S, NO)


# revision 5
# speedup vs baseline: 1.0115x; 1.0115x over previous
"""ConcatenatedLoRALinearSidecarLayer kernel for 8x TRN2 NeuronCores.

Reference computation (per LoRA branch n, then concat over n on the last dim):
    h_n = x @ down_n.T                      # [M, R]
    y_n = (h_n @ up_n.T + bias_n) * (WEIGHT * scales_n)
    out = concat_n(y_n)                     # [M, N*O]

Strategy (v2 -- DMA-roofline oriented):
  - Data-parallel over tokens M = B*S = 16384 -> 2048 tokens per core.
  - ALL device IO in bf16: x in, down/up weights in, y out. The fp32 result
    is reconstructed on the host (bf16 -> fp32 upcast + bias add). This
    roughly halves HBM traffic vs fp32 (the old roofline): per core
    ~17MB x-in + ~50MB y-out + ~6MB weights ~= 73MB @ ~358 GB/s ~= 205us.
  - bias*weight*scale is folded on the host and added on the HOST, not the
    device: a PSUM-sourced tensor_tensor add runs at 1x DVE mode and was
    costing ~250us/core in the old kernel. Device only ever does
    tensor_copy / activation-Copy out of PSUM.
  - Host-side prep lays x out so every x tile DMA is fully contiguous
    per partition (16KB lines, 4MB transfers); y writes are full token
    rows (24KB lines, 3MB transfers).
  - Per core, per 512-token block:
      phase 1:  hT_n[r, t] += dT_n[d, r].T @ xT[d, t] over 32 d-chunks
                (moving free dim 512, fp32 PSUM accumulate)
      hT copy:  DVE PSUM->SBUF, cast to bf16
      phase 2:  per 128-token sub-block, per branch, per 512-wide o chunk:
                y[t, o] = hT_n[r, t].T @ uT_n[r, o]  (single MM, fp32 PSUM)
      copyback: PSUM->SBUF bf16 casts alternate between DVE tensor_copy
                and ScalarE activation(Copy) so neither engine bottlenecks.
  - All weights (dT, uT) stay resident in SBUF.

Wait-slot legalization: this container's walrus accepts at most 1 sync-wait
per instruction; a JSON post-pass splits excess waits onto same-engine NoOps
with identical blocking semantics.
"""

from contextlib import ExitStack

import numpy as np

import concourse.bass as bass
import concourse.mybir as mybir
import concourse.tile as tile

WEIGHT = 0.8
N_CORES = 8
B, S, D = 4, 4096, 4096
NL, R, O = 3, 128, 4096
M = B * S                    # 16384 tokens total
T = M // N_CORES             # 2048 tokens per core
NR = NL * R                  # 384
NO = NL * O                  # 12288

P = 128                      # SBUF partitions
TB = 512                     # token block (phase-1 moving free dim)
DO = D // P                  # 32 contraction chunks
DH = DO // 2                 # d-chunks per x half-load
OC = 512                     # phase-2 moving free dim / PSUM tile (fp32 bank)

F32 = mybir.dt.float32
BF16 = mybir.dt.bfloat16

MAX_WAITS = 1


def build_nc(t_core: int = T) -> bass.Bass:
    assert t_core % TB == 0
    n_tb = t_core // TB

    nc = bass.Bass("TRN2", target_bir_lowering=False, debug=False)

    xT = nc.dram_tensor("xT", [P, n_tb, 2, DH, TB], BF16, kind="ExternalInput")
    dT = nc.dram_tensor("dT", [P, DO, NR], BF16, kind="ExternalInput")
    uT = nc.dram_tensor("uT", [R, NO], BF16, kind="ExternalInput")
    y = nc.dram_tensor("y", [t_core, NO], BF16, kind="ExternalOutput")

    copy_fn = mybir.ActivationFunctionType.Copy

    with tile.TileContext(nc) as tc, ExitStack() as ctx:
        const = ctx.enter_context(tc.tile_pool(name="const", bufs=1))
        xpool = ctx.enter_context(tc.tile_pool(name="xpool", bufs=4))
        hpool = ctx.enter_context(tc.tile_pool(name="hpool", bufs=2))
        ypool = ctx.enter_context(tc.tile_pool(name="ypool", bufs=3))
        ps_h = ctx.enter_context(tc.tile_pool(name="ps_h", bufs=3, space="PSUM"))
        ps_y = ctx.enter_context(tc.tile_pool(name="ps_y", bufs=4, space="PSUM"))

        # Resident weights. dT comes in two halves interleaved with block-0's
        # x halves so the first matmul only waits on ~5.5MB; uT (first needed
        # by phase 2, ~40us in) is queued after all of block-0's x.
        dT_sb = const.tile([P, DO, NR], BF16, name="dT_sb")
        uT_sb = const.tile([P, NO], BF16, name="uT_sb")
        nc.sync.dma_start(dT_sb[:, :DH, :], dT[:, :DH, :])

        x_tiles = {}
        for tb in range(n_tb):
            for h in range(2):
                x_tiles[(tb, h)] = xpool.tile(
                    [P, DH, TB], BF16, tag="xt", name=f"xt{tb}_{h}"
                )

        nc.sync.dma_start(x_tiles[(0, 0)][:], xT[:, 0, 0, :, :])
        nc.sync.dma_start(dT_sb[:, DH:, :], dT[:, DH:, :])
        nc.sync.dma_start(x_tiles[(0, 1)][:], xT[:, 0, 1, :, :])
        nc.sync.dma_start(uT_sb[:], uT[:, :])

        for tb in range(n_tb):
            xts = [x_tiles[(tb, 0)], x_tiles[(tb, 1)]]
            # Prefetch the NEXT block's x now, ahead of this block's y-write
            # DMAs on the (FIFO) SP queue.
            if tb + 1 < n_tb:
                for h in range(2):
                    nc.sync.dma_start(
                        x_tiles[(tb + 1, h)][:], xT[:, tb + 1, h, :, :]
                    )

            # Phase 1: hT_n[r, 0:TB] accumulated over all d chunks.
            hps = [
                ps_h.tile([P, TB], F32, tag="hps", name=f"hps{tb}_{n}")
                for n in range(NL)
            ]
            for dc in range(DO):
                xs = xts[dc // DH][:, dc % DH, :]
                for n in range(NL):
                    nc.tensor.matmul(
                        hps[n][:],
                        dT_sb[:, dc, n * R:(n + 1) * R],
                        xs,
                        start=(dc == 0),
                        stop=(dc == DO - 1),
                    )

            hT = hpool.tile([P, NL, TB], BF16, tag="hT", name=f"hT{tb}")
            for n in range(NL):
                nc.vector.tensor_copy(hT[:, n, :], hps[n][:])

            # Phase 2: y[t, o] per 128-token sub-block, per branch, per chunk.
            for th in range(TB // P):
                t0 = tb * TB + th * P
                ysb = ypool.tile([P, NO], BF16, tag="ysb", name=f"ysb{tb}_{th}")
                k = 0
                for n in range(NL):
                    lhs = hT[:, n, th * P:(th + 1) * P]
                    o0 = n * O
                    for oc in range(O // OC):
                        yps = ps_y.tile([P, OC], F32, tag="yps",
                                        name=f"yps{tb}_{th}_{n}_{oc}")
                        nc.tensor.matmul(
                            yps[:],
                            lhs,
                            uT_sb[:, o0 + oc * OC: o0 + (oc + 1) * OC],
                            start=True,
                            stop=True,
                        )
                        dst = ysb[:, o0 + oc * OC: o0 + (oc + 1) * OC]
                        if k % 2 == 0:
                            nc.vector.tensor_copy(dst, yps[:])
                        else:
                            nc.scalar.activation(dst, yps[:], copy_fn)
                        k += 1
                        if k == 12:
                            nc.sync.dma_start(
                                y[t0:t0 + P, :NO // 2], ysb[:, :NO // 2]
                            )
                nc.sync.dma_start(y[t0:t0 + P, NO // 2:], ysb[:, NO // 2:])

    _wrap_to_json_with_wait_split(nc)
    return nc


def _legalize_wait_counts(bir: dict) -> None:
    """Split multi-wait instructions: this walrus accepts only ONE sync-wait
    per instruction. Excess waits move onto NoOps inserted just before the
    instruction on the same engine -- identical blocking semantics."""
    n_new = 0
    for fn in bir.get("functions", []):
        for blk in fn.get("blocks", []):
            insts = blk.get("instructions", [])
            out = []
            for inst in insts:
                si = inst.get("sync_info")
                waits = (si or {}).get("on_wait") or []
                if len(waits) > MAX_WAITS:
                    for w in waits[:-1]:
                        nonlocal_name = f"I-waitsplit-{id(inst)}-{n_new}"
                        n_new += 1
                        out.append({
                            "debug": inst.get("debug", 0),
                            "engine": inst["engine"],
                            "ins": [],
                            "name": nonlocal_name,
                            "opcode": "NoOp",
                            "outs": [],
                            "sync_info": {"on_update": [], "on_wait": [w]},
                        })
                    si["on_wait"] = [waits[-1]]
                out.append(inst)
            blk["instructions"] = out


def _wrap_to_json_with_wait_split(nc) -> None:
    import json as _json

    orig = nc.to_json_bytes

    def patched():
        d = _json.loads(orig())
        _legalize_wait_counts(d)
        return _json.dumps(d).encode()

    nc.to_json_bytes = patched


def prep_inputs(x, down, up, bias, scales):
    """Host-side marshalling: transpose + fold scales + bf16 casts.

    Returns (per-core in_maps, bias_w) where bias_w is the host-side
    fp32 bias (already scaled) to add after the device run.
    """
    import ml_dtypes

    x = np.asarray(x, dtype=np.float32)
    down = np.asarray(down, dtype=np.float32)
    up = np.asarray(up, dtype=np.float32)
    bias = np.asarray(bias, dtype=np.float32)
    scales = np.asarray(scales, dtype=np.float32)

    ws = (WEIGHT * scales).astype(np.float32)                       # [NL]
    bias_w = (bias * ws[:, None]).reshape(NO).astype(np.float32)    # [NO]

    # down [NL,R,D] -> [D, NR] -> [do, di, NR] -> [di=128, do, NR]
    dTf = np.ascontiguousarray(
        np.transpose(down, (2, 0, 1)).reshape(DO, P, NR).transpose(1, 0, 2)
    ).astype(ml_dtypes.bfloat16)
    # up [NL,O,R] * ws -> [R, NO]
    uTf = np.ascontiguousarray(
        np.transpose(up * ws[:, None, None], (2, 0, 1)).reshape(R, NO)
    ).astype(ml_dtypes.bfloat16)

    # x -> [D, M] -> per-core [di, tb, half, dh, t] fully-contiguous tiles
    xTf = np.ascontiguousarray(x.reshape(M, D).T).astype(ml_dtypes.bfloat16)

    n_tb = T // TB
    in_maps = []
    for c in range(N_CORES):
        xc = xTf[:, c * T:(c + 1) * T]                    # [D, T]
        xc = xc.reshape(2, DH, P, n_tb, TB)               # [half, dh, di, tb, t]
        xc = np.ascontiguousarray(xc.transpose(2, 3, 0, 1, 4))  # [di,tb,h,dh,t]
        in_maps.append({
            "xT": xc,
            "dT": dTf,
            "uT": uTf,
        })
    return in_maps, bias_w


_CACHED_NC = None


def kernel(x, down, up, bias, scales):
    global _CACHED_NC
    from concourse.bass_utils import run_bass_kernel_spmd

    in_maps, bias_w = prep_inputs(x, down, up, bias, scales)
    if _CACHED_NC is None:
        _CACHED_NC = build_nc(T)
    res = run_bass_kernel_spmd(_CACHED_NC, in_maps, core_ids=list(range(N_CORES)))
    out = np.concatenate(
        [np.asarray(r["y"]).astype(np.float32) for r in res.results], axis=0
    )
    out += bias_w[None, :]
    return out.reshape(B, S, NO)
